# revision 3
# baseline (speedup 1.0000x reference)
"""LCGP prediction kernel for Trainium2, sharded over 8 NeuronCores.

Strategy (expert-parallel over the q=8 GP components, one per core):
  Per core q:
    C0T[n, m] = exp(lLmb0[q] - ||a_m - b_n||^2)   (fused PE matmul over
        hi/lo-split fp16 feature rows, ACT exp -> fp16 chunk, DVE -> fp8)
    t = C0 @ Th[q]           (the big [2048,4096]x[4096,4096] GEMM, fp8
                              DoubleRow on PE: K=256 per pass, fp32 PSUM)
    sumt2[m] = sum_r t[m,r]^2   (fused ACT Square+accum_out on PSUM tiles)
    ghat[m]  = C0 @ CinvM[q]    (fp16 matmuls over the exact C0 chunks,
                                 accumulated in PSUM during phase 1)
  Host: tiny [q,n0] -> [p,n0] psi projection in fp32 numpy.

Th is pre-scaled (x64) and pre-interleaved to the DoubleRow pair layout
[ku, 128, 2, n] in fp8 on the host, so it streams from HBM once per
m-half (2 x 16 MB/core) with no on-device conversion.  The n0 axis is
split in two halves so half h+1's C0T (exp/convert/ghat) work overlaps
half h's GEMM: one phase-1 unit is interleaved after each (j, i) GEMM
iteration.  Cost model: ~300 us/core vs 952 us for the fp16 GEMM version.
"""

import os

import numpy as np

import concourse.bacc as bacc
import concourse.bass as bass
import concourse.mybir as mybir
import concourse.tile as tile

P = 128
FP32 = mybir.dt.float32
FP16 = mybir.dt.float16
FP8 = mybir.dt.float8e4
DR = mybir.MatmulPerfMode.DoubleRow

# Full-size problem dims (hardcoded per spec: q=8, d=8, p=64, n=4096, n0=2048)
Q_FULL = 8
N_FULL = 4096
N0_FULL = 2048

TH_SCALE = 64.0   # Th is ~N(0, 0.01^2); x64 keeps fp8 e4m3 in normal range


def build_nc(n=N_FULL, n0=N0_FULL, rb=512, nh=2, mc=512, fk=32, debug=False):
    """Build the single-core Bass program (same program on all 8 cores)."""
    kt = n // P            # 32 contraction k-tiles of 128
    ku = kt // 2           # 16 DoubleRow pair-tiles (K=256 each)
    mh = n0 // nh          # rows per m-half
    mt = mh // P           # m-tiles per half
    nmc = mh // mc         # phase-1 chunks per half
    nrb = n // rb          # r blocks of the big GEMM

    nc = bacc.Bacc("TRN2", target_bir_lowering=False, debug=debug)

    a_feat = nc.dram_tensor("a_feat", [fk, n0], FP16, kind="ExternalInput")
    b_feat = nc.dram_tensor("b_feat", [fk, n], FP16, kind="ExternalInput")
    th8 = nc.dram_tensor("th8", [ku, P, 2, n], FP8, kind="ExternalInput")
    cinv = nc.dram_tensor("cinv", [P, kt], FP16, kind="ExternalInput")
    ghat_o = nc.dram_tensor("ghat", [n0 // mc, mc], FP32, kind="ExternalOutput")
    sumt2_o = nc.dram_tensor("sumt2", [n0 // P, P], FP32, kind="ExternalOutput")

    with tile.TileContext(nc) as tc:
        with (
            tc.tile_pool(name="feat", bufs=1) as featp,
            tc.tile_pool(name="c0t", bufs=2 * ku) as c0tp,
            tc.tile_pool(name="thsb", bufs=2 * ku) as thsbp,
            tc.tile_pool(name="gsb", bufs=2 * mt) as gsbp,
            tc.tile_pool(name="scr", bufs=4) as scrp,
            tc.tile_pool(name="sqps", bufs=2, space=bass.MemorySpace.PSUM) as sqpsp,
            tc.tile_pool(name="tps", bufs=4, space=bass.MemorySpace.PSUM) as tpsp,
            tc.tile_pool(name="ghps", bufs=2, space=bass.MemorySpace.PSUM) as ghpsp,
        ):
            bf = featp.tile([fk, n], FP16, tag="bf")
            af = featp.tile([fk, n0], FP16, tag="af")
            cv = featp.tile([P, kt], FP16, tag="cv")
            nc.sync.dma_start(bf[:], b_feat[:])
            nc.sync.dma_start(af[:], a_feat[:])
            nc.sync.dma_start(cv[:], cinv[:])

            # fp8 C0T pair-tiles [128, 2, mh]: [:, e, m] = C0T[u*256+e*128+p, m]
            c0t = {h: [c0tp.tile([P, 2, mh], FP8, tag="c0t", name=f"c0t_{h}_{u}")
                       for u in range(ku)]
                   for h in range(nh)}
            ghacc = {}

            def phase1_unit(h, c, k):
                """One (chunk, k-tile) of C0T for half h: sq matmul, exp,
                fp8 downcast, and the fp16 ghat partial matmul."""
                ps = sqpsp.tile([P, mc], FP32, tag="sqps", name=f"ps_{h}_{c}_{k}")
                nc.tensor.matmul(
                    ps[:],
                    bf[:, k * P:(k + 1) * P],
                    af[:, h * mh + c * mc: h * mh + (c + 1) * mc],
                    start=True, stop=True,
                )
                sc1 = scrp.tile([P, mc], FP16, tag="sc1", name=f"sc1_{h}_{c}_{k}")
                nc.scalar.activation(
                    sc1[:], ps[:],
                    mybir.ActivationFunctionType.Exp,
                    bias=0.0, scale=-1.0,
                )
                nc.vector.tensor_copy(
                    c0t[h][k // 2][:, k % 2, c * mc:(c + 1) * mc], sc1[:])
                if k == 0:
                    ghacc[h, c] = ghpsp.tile([1, mc], FP32, tag="ghps",
                                             name=f"ghacc_{h}_{c}")
                nc.tensor.matmul(
                    ghacc[h, c][:], cv[:, k:k + 1], sc1[:],
                    start=(k == 0), stop=(k == kt - 1),
                    skip_group_check=True,
                )
                if k == kt - 1:
                    gh = gsbp.tile([1, mc], FP32, tag="ghsb", bufs=2)
                    nc.vector.tensor_copy(gh[:], ghacc[h, c][:])
                    nc.sync.dma_start(ghat_o[h * nmc + c, :], gh[:])

            def phase2_half(h, units):
                """Big GEMM for half h; `units` = phase-1 work for half h+1,
                one unit interleaved after each (j, i) iteration."""
                gaccs = [gsbp.tile([P, nrb], FP32, tag="gacc", name=f"gacc_{h}_{i}")
                         for i in range(mt)]
                ui = iter(units)
                for j in range(nrb):
                    slabs = []
                    for u in range(ku):
                        st = thsbp.tile([P, 2, rb], FP8, tag="thsb",
                                        name=f"st_{h}_{j}_{u}")
                        nc.sync.dma_start(st[:], th8[u, :, :, j * rb:(j + 1) * rb])
                        slabs.append(st)
                    for i in range(mt):
                        tp = tpsp.tile([P, rb], FP32, tag="tps",
                                       name=f"tp_{h}_{j}_{i}")
                        for u in range(ku):
                            nc.tensor.matmul(
                                tp[:], c0t[h][u][:, :, i * P:(i + 1) * P],
                                slabs[u][:],
                                start=(u == 0), stop=(u == ku - 1),
                                perf_mode=DR, skip_group_check=True,
                            )
                        # fused square + row-sum of the t tile on ACT
                        sc = scrp.tile([P, rb], FP16, tag="scr", bufs=2,
                                       name=f"sc_{h}_{j}_{i}")
                        nc.scalar.activation(
                            sc[:], tp[:], mybir.ActivationFunctionType.Square,
                            bias=0.0, scale=1.0 / TH_SCALE,
                            accum_out=gaccs[i][:, j:j + 1],
                        )
                        nu = next(ui, None)
                        if nu is not None:
                            phase1_unit(*nu)
                for nu in ui:
                    phase1_unit(*nu)
                for i in range(mt):
                    s2 = gsbp.tile([P, 1], FP32, tag="s2", bufs=4)
                    nc.vector.tensor_reduce(
                        s2[:], gaccs[i][:], axis=mybir.AxisListType.X,
                        op=mybir.AluOpType.add)
                    nc.sync.dma_start(sumt2_o[h * mt + i, :], s2[:])

            units = {h: [(h, c, k) for c in range(nmc) for k in range(kt)]
                     for h in range(nh)}
            for u in units[0]:
                phase1_unit(*u)
            for h in range(nh):
                phase2_half(h, units[h + 1] if h + 1 < nh else [])

    nc.compile()
    return nc


def _features_for_q(x0s, x, inv_l_q, lLmb0_q, fk=32):
    """Host prep: hi/lo-split fp16 feature rows so the PE computes
    sq_mod[n, m] = ||a_m - b_n||^2 - lLmb0 in near-fp32 precision."""
    f16, f32 = np.float16, np.float32
    a = (x0s * inv_l_q).astype(f32)            # [n0, d]
    b = (x * inv_l_q).astype(f32)              # [n, d]
    sqa = (a * a).sum(-1, dtype=f32) - f32(lLmb0_q)
    sqb = (b * b).sum(-1, dtype=f32)

    def hilo(v):
        hi = v.astype(f16)
        lo = (v - hi.astype(f32)).astype(f16)
        return hi, lo

    a_hi, a_lo = hilo(a)
    b_hi, b_lo = hilo(b)
    sqa_hi, sqa_lo = hilo(sqa)
    sqb_hi, sqb_lo = hilo(sqb)
    d = a.shape[1]
    n0, n = a.shape[0], b.shape[0]
    assert 3 * d + 4 <= fk
    af = np.zeros((fk, n0), f16)
    bf = np.zeros((fk, n), f16)
    m2a_hi = (-2.0 * a_hi.astype(f32)).astype(f16).T   # exact in fp16
    m2a_lo = (-2.0 * a_lo.astype(f32)).astype(f16).T
    af[0:d] = m2a_hi
    af[d:2 * d] = m2a_hi
    af[2 * d:3 * d] = m2a_lo
    af[3 * d] = sqa_hi
    af[3 * d + 1] = sqa_lo
    af[3 * d + 2] = 1.0
    af[3 * d + 3] = 1.0
    bf[0:d] = b_hi.T
    bf[d:2 * d] = b_lo.T
    bf[2 * d:3 * d] = b_hi.T
    bf[3 * d] = 1.0
    bf[3 * d + 1] = 1.0
    bf[3 * d + 2] = sqb_hi
    bf[3 * d + 3] = sqb_lo
    return af, bf


def prep_core_inputs(inputs, q, fk=32):
    """Per-core (per-component) input map for the device kernel."""
    f16, f32 = np.float16, np.float32
    f8 = mybir.dt.np(FP8)
    x0 = np.asarray(inputs["x0"], f32)
    x = np.asarray(inputs["x"], f32)
    x_min = np.asarray(inputs["x_min"], f32)
    x_max = np.asarray(inputs["x_max"], f32)
    lLmb = np.asarray(inputs["lLmb"], f32)
    lLmb0 = np.asarray(inputs["lLmb0"], f32)
    x0s = (x0 - x_min) / (x_max - x_min)
    inv_l = np.exp(-0.5 * lLmb[q]).astype(f32)
    af, bf = _features_for_q(x0s, x, inv_l, lLmb0[q], fk=fk)

    n = x.shape[0]
    ku = n // (2 * P)
    # DoubleRow pair layout: th8[u, p, e, r] = 64*Th[q][u*256 + e*128 + p, r]
    th = np.asarray(inputs["Th"], f32)[q]
    th8 = np.ascontiguousarray(
        (th * TH_SCALE).reshape(ku, 2, P, n).swapaxes(1, 2)).astype(f8)

    cinv = np.asarray(inputs["CinvM"], f32)[q].astype(f16)
    cinv_t = np.ascontiguousarray(cinv.reshape(n // P, P).T)   # [128, kt]
    return {"a_feat": af, "b_feat": bf, "th8": th8, "cinv": cinv_t}


def finish_host(inputs, ghat_all, sumt2_all):
    """Final tiny [q,n0] -> [p,n0] projection, fp32 on host (mirrors reference)."""
    f32 = np.float32
    lLmb0 = np.asarray(inputs["lLmb0"], f32)
    lnug = np.asarray(inputs["lnugGPs"], f32)
    lsig = np.asarray(inputs["lsigma2s"], f32)
    phi = np.asarray(inputs["phi"], f32)
    ystd = np.asarray(inputs["ystd"], f32)
    ymean = np.asarray(inputs["ymean"], f32)

    c00 = (np.exp(lLmb0) * (1.0 + np.exp(lnug))).astype(f32)[:, None]
    gvar = c00 - sumt2_all                        # [q, n0]
    sig = np.exp(lsig).astype(f32)                # [p]
    psi = (phi * np.sqrt(sig)[:, None]).astype(f32)
    predmean = (psi @ ghat_all).astype(f32)       # [p, n0]
    confvar = (gvar.T @ (psi ** 2).T).astype(f32)  # [n0, p]
    predvar = confvar + sig
    ypred = (predmean * ystd + ymean).astype(f32)
    yconfvar = (confvar.T * ystd ** 2).astype(f32)
    ypredvar = (predvar.T * ystd ** 2).astype(f32)
    return ypred, ypredvar, yconfvar


_NC_CACHE = {}
LAST_RESULTS = None


def kernel(**inputs):
    from concourse.bass_utils import run_bass_kernel_spmd

    global LAST_RESULTS
    q_n = Q_FULL
    n0 = N0_FULL

    if "nc" not in _NC_CACHE:
        _NC_CACHE["nc"] = build_nc()
    nc = _NC_CACHE["nc"]

    in_maps = [prep_core_inputs(inputs, q) for q in range(q_n)]
    core_ids = list(range(q_n))
    res = run_bass_kernel_spmd(
        nc, in_maps, core_ids,
        trace=bool(os.environ.get("LCGP_TRACE")),
    )
    LAST_RESULTS = res

    ghat_all = np.zeros((q_n, n0), np.float32)
    sumt2_all = np.zeros((q_n, n0), np.float32)
    for q in range(q_n):
        ghat_all[q] = np.asarray(res.results[q]["ghat"]).reshape(n0)
        sumt2_all[q] = np.asarray(res.results[q]["sumt2"]).reshape(n0)

    return finish_host(inputs, ghat_all, sumt2_all)
